# revision 35
# baseline (speedup 1.0000x reference)
"""GCN (CapsGNN) message-passing kernel for 8 Trainium2 NeuronCores.

Algorithm (mathematically identical to the reference):
    deg[i] = 1 + indeg(i);  dis = deg**-0.5
    With xt := dis * x (row-scaled activations), each layer is
        y = relu( dis[d] * ( sum_{e: dst=d} xt[src_e]  +  xt[d] ) @ W + b )
    because norm(e) = dis[src]*dis[dst] is separable and matmul is linear.

Distribution: nodes are sharded contiguously across 8 cores (core c owns
rows [c*P, (c+1)*P)); edges are partitioned by destination shard so the
segment-sum is local.  Tables and all matmul operands are bf16 (validated
rel err ~8e-3 vs the 2e-2 gate).  Per layer each core:
  1. dma_gather's xt[src] rows (256B bf16 each) for its edges from a full
     replicated table in HBM (int16 gather indices -> table is addressed
     in two chunk-aligned halves of < 32768 rows each),
  2. accumulates the segment-sum TRANSPOSED in PSUM: for each 128-edge
     tile, matmul(psT[f,d] += msg[e,f].T @ onehot[e,d]) with on-chip
     one-hot matrices (iota == dst_local), plus one identity matmul that
     adds the self-loop term xt_local.T -- so no per-block transpose is
     needed before the weight matmul,
  3. epilogue per 128-dst block: uT = psT * disrow (free-dim dis scale,
     one DVE op), y = uT.T @ W (PE), relu (ACT), xn = y * dis (DVE),
  4. writes its shard of the next table and AllGathers it across cores.

Host-side preprocessing (free): degree computation, edge sorting/padding
by (dst block, src half, src), gather indices, dst-local labels.
"""

import math
import numpy as np

N_CORES = 8
F = 128  # feature width of every hidden layer (== partition count)
BLK = 128  # dst nodes per aggregation block
# Max 128-idx tiles per dma_gather call: the SWDGE ring is hard-capped at
# 1024 descriptors per call (GCAP=10/16/18 hang the device; scratch size
# does not change it).
GCAP = 8
DMA_SCRATCH = 16384


# --------------------------------------------------------------------------
# Host-side preprocessing
# --------------------------------------------------------------------------

def _preprocess(features, W, b, W_out, b_out, edges):
    import ml_dtypes
    bf16 = ml_dtypes.bfloat16

    features = np.asarray(features, dtype=np.float32)
    W = np.asarray(W, dtype=np.float32)
    b = np.asarray(b, dtype=np.float32)
    W_out = np.asarray(W_out, dtype=np.float32)
    b_out = np.asarray(b_out, dtype=np.float32)
    edges = np.asarray(edges)

    N = features.shape[0]
    C = N_CORES
    assert N % C == 0
    P = N // C
    NB = (P + BLK - 1) // BLK
    E = edges.shape[1]

    src = edges[0].astype(np.int64)
    dst = edges[1].astype(np.int64)

    deg = (np.bincount(dst, minlength=N).astype(np.float32) + np.float32(1.0))
    dis = (deg ** np.float32(-0.5)).astype(np.float32)
    xt0 = (features * dis[:, None]).astype(bf16)

    core = dst // P
    blk = (dst % P) // BLK
    dloc = ((dst % P) % BLK).astype(np.float32)

    # ---- chunk-major table layout -------------------------------------
    # The full gather table is stored chunk-major: for chunk k (a range of
    # dst blocks), all cores' rows of that chunk are contiguous.  This lets
    # the per-layer AllGather run as NCHUNK slice-collectives, each firing
    # as soon as its blocks' outputs are written (overlapped with compute).
    nch = 4
    base_bl = NB // nch
    extra = NB - base_bl * nch
    chunk_blocks = [base_bl + (1 if k < extra else 0) for k in range(nch)]
    bl_start = np.concatenate([[0], np.cumsum(chunk_blocks)]).astype(np.int64)
    row_start = np.minimum(bl_start * BLK, P)  # per-core local row ranges
    chunk_rows = np.diff(row_start)            # rows per core per chunk
    gbase = np.concatenate([[0], np.cumsum(chunk_rows * C)]).astype(np.int64)

    rloc_all = np.arange(N, dtype=np.int64) % P
    core_all = np.arange(N, dtype=np.int64) // P
    k_all = np.searchsorted(row_start[1:], rloc_all, side="right")
    posv = gbase[k_all] + core_all * chunk_rows[k_all] + (rloc_all - row_start[k_all])
    xt0_perm = np.empty_like(xt0)
    xt0_perm[posv] = xt0

    cs = int(gbase[2])  # first-two-chunks size; both halves fit int16
    assert cs < 32768 and N - cs < 32768
    psrc = posv[src]
    half = (psrc >= cs).astype(np.int64)
    idxv = (psrc - half * cs).astype(np.int16)

    # group id per edge: (core, blk, half)
    gid = (core * NB + blk) * 2 + half
    cnt = np.bincount(gid, minlength=C * NB * 2).reshape(C, NB, 2)
    Tneed = -(-cnt // BLK)  # ceil division -> tiles needed per group
    T = Tneed.max(axis=0)  # [NB, 2] same tile counts on every core (SPMD)
    for bl in range(NB):
        if T[bl].sum() == 0:
            T[bl, 0] = 1  # keep at least one (all-padding) tile per block

    off = np.zeros((NB, 2), np.int64)
    o = 0
    for bl in range(NB):
        for h in (0, 1):
            off[bl, h] = o
            o += T[bl, h]
    NT = int(o)

    # slab position for every edge: off[blk,half]*128 + within-group rank.
    # Secondary sort by src position keeps gather addresses ascending within
    # a group (HBM row locality).
    order = np.lexsort((psrc, half, blk, core))
    counts_flat = np.bincount(gid, minlength=C * NB * 2)
    starts = np.zeros(C * NB * 2, np.int64)
    np.cumsum(counts_flat[:-1], out=starts[1:])
    rank = np.arange(E, dtype=np.int64) - starts[gid[order]]
    soff = off[blk[order], half[order]] * BLK + rank

    idx_slab = np.zeros((C, NT * BLK), np.int16)
    dl_slab = np.full((C, NT * BLK), -1.0, np.float32)  # cast bf16 below
    cc = core[order]
    idx_slab[cc, soff] = idxv[order]
    dl_slab[cc, soff] = dloc[order]

    iota = np.ascontiguousarray(
        np.broadcast_to(np.arange(BLK, dtype=np.float32), (BLK, BLK))
    ).astype(bf16)
    ident = np.eye(BLK, dtype=np.float32).astype(bf16)
    has_bias = bool(np.any(b != 0.0))
    NL = W.shape[0]  # stacked hidden layers (3)

    # ---- final (128->1) layer: fixed-K scalar slot table ------------------
    # out[d] = relu(dis_d * (sum_{e->d} s[src_e] + s[d]) + b_out) with
    # s[n] = xt3[n] @ W_out, so the last layer only needs per-edge SCALARS.
    # s (bf16, [N]) is AllGather'd (200KB) and replicated across SBUF
    # partitions; per dst-block a single gpsimd indirect_copy fetches
    # s[src] for every edge into a [128, 16, K] slab (lane p sums range
    # p%16 of its 16-partition group), padded per dst to the global max
    # in-degree K with pointers to a zero slot.
    indeg = (deg - np.float32(1.0)).astype(np.int64)
    KSLOT = int(indeg.max())
    order2 = np.lexsort((src, dst))
    dsto = dst[order2]
    srco = src[order2]
    starts2 = np.searchsorted(dsto, np.arange(N, dtype=np.int64))
    rank2 = np.arange(E, dtype=np.int64) - starts2[dsto]
    assert rank2.max() < KSLOT
    c2 = dsto // P
    pl2 = dsto % P
    bl2 = pl2 // BLK
    dl2 = pl2 % BLK
    i2 = (dl2 % 16) * KSLOT + rank2      # slot within the 16-lane group
    g2 = dl2 // 16
    idxf = np.full((C, 128, NB * KSLOT), N, np.int64)  # pad -> zero slot N
    idxf[c2, 16 * g2 + (i2 % 16), bl2 * KSLOT + i2 // 16] = srco
    ddiag = (np.arange(16)[None, :] ==
             (np.arange(128) % 16)[:, None]).astype(np.float32)

    plan = dict(N=N, C=C, P=P, NB=NB, NT=NT, CS=cs, NL=NL, KSLOT=KSLOT,
                T=T.tolist(), off=off.tolist(),
                Tmax=int(T.max()), has_bias=has_bias,
                bl_start=bl_start.tolist(), row_start=row_start.tolist(),
                gbase=gbase.tolist(), nch=nch)

    in_maps = []
    for c in range(C):
        dis_c = np.zeros(NB * BLK, np.float32)
        dis_c[:P] = dis[c * P:(c + 1) * P]
        m = {
            "xt0": xt0_perm,
            "xtl0": np.ascontiguousarray(xt0[c * P:(c + 1) * P]),
            "idx": np.ascontiguousarray(
                np.tile(idx_slab[c].reshape(NT * 8, 16).T, (8, 1))),
            # pair-duplicated dstloc: dld[p, 2t+r] = dloc of edge (t, p),
            # r=0,1.  The duplicate lets the batched one-hot comparison keep
            # a [stride 1, count 2] innermost AP dim (DVE 2x_1p fast mode).
            "dstloc": np.ascontiguousarray(np.repeat(
                dl_slab[c].reshape(NT, BLK).T, 2, axis=1)).astype(bf16),
            "discol": np.ascontiguousarray(dis_c.reshape(NB, BLK).T),
            # dis broadcast down partitions: [128, NB*128] for free-dim scale
            "disrow": np.ascontiguousarray(
                np.broadcast_to(dis_c[None, :], (BLK, NB * BLK))).astype(bf16),
            "iota": iota,
            "ident": ident,
            "wout": np.ascontiguousarray(W_out).astype(bf16),
            "boutc": np.full((BLK, 1), b_out[0], np.float32),
        }
        for l in range(NL):
            m[f"w{l}"] = np.ascontiguousarray(W[l]).astype(bf16)
            if has_bias:
                m[f"bb{l}"] = np.ascontiguousarray(
                    np.broadcast_to(b[l][None, :], (BLK, F))).astype(np.float32)
        in_maps.append(m)
    return plan, in_maps


# --------------------------------------------------------------------------
# Bass/Tile kernel builder
# --------------------------------------------------------------------------

def _build(plan):
    import os
    from concourse import bacc, tile
    import concourse.mybir as mybir

    abl = os.environ.get("KABL", "")  # ablation probe: noag/nogather/noonehot/nomm
    reps = int(plan.get("reps", 1))  # whole-network repetitions (timing slope)

    N, C, P, NB, NT, CS, NL, KSLOT = (plan[k] for k in
                                      ("N", "C", "P", "NB", "NT", "CS", "NL",
                                       "KSLOT"))
    hb = [0, CS, N]  # gather-table half bounds == chunk collective regions
    T, off, Tmax, has_bias = (plan[k] for k in
                              ("T", "off", "Tmax", "has_bias"))
    bl_start, row_start, gbase, nch = (plan[k] for k in
                                       ("bl_start", "row_start", "gbase",
                                        "nch"))
    f32 = mybir.dt.float32
    bf16 = mybir.dt.bfloat16
    i16 = mybir.dt.int16
    u16 = mybir.dt.uint16
    Relu = mybir.ActivationFunctionType.Relu
    eq = mybir.AluOpType.is_equal
    add = mybir.AluOpType.add
    mult = mybir.AluOpType.mult

    nc = bacc.Bacc("TRN2", debug=False, num_devices=C,
                   target_bir_lowering=False,
                   dynamic_dma_scratch_size=DMA_SCRATCH)

    xt0_d = nc.dram_tensor("xt0", [N, F], bf16, kind="ExternalInput")
    xtl0_d = nc.dram_tensor("xtl0", [P, F], bf16, kind="ExternalInput")
    idx_d = nc.dram_tensor("idx", [128, NT * 8], i16, kind="ExternalInput")
    dl_d = nc.dram_tensor("dstloc", [128, NT * 2], bf16, kind="ExternalInput")
    disc_d = nc.dram_tensor("discol", [128, NB], f32, kind="ExternalInput")
    disr_d = nc.dram_tensor("disrow", [128, NB * BLK], bf16,
                            kind="ExternalInput")
    iota_d = nc.dram_tensor("iota", [128, 128], bf16, kind="ExternalInput")
    ident_d = nc.dram_tensor("ident", [128, 128], bf16, kind="ExternalInput")
    w_d = [nc.dram_tensor(f"w{l}", [F, F], bf16, kind="ExternalInput")
           for l in range(NL)]
    wout_d = nc.dram_tensor("wout", [F, 1], bf16, kind="ExternalInput")
    boutc_d = nc.dram_tensor("boutc", [128, 1], f32, kind="ExternalInput")
    bb_d = [nc.dram_tensor(f"bb{l}", [128, F], f32, kind="ExternalInput")
            for l in range(NL)] if has_bias else None
    out_d = nc.dram_tensor("out", [P, 1], f32, kind="ExternalOutput")

    shard = [nc.dram_tensor(f"xsh{l}", [P, F], bf16) for l in range(NL)]
    full = [nc.dram_tensor(f"xfull{l}", [N, F], bf16, addr_space="Shared")
            for l in range(NL)]

    with tile.TileContext(nc) as tc:
        with (
            tc.tile_pool(name="const", bufs=1) as cpool,
            tc.tile_pool(name="msg", bufs=4) as mpool,
            tc.tile_pool(name="oh", bufs=4) as ohpool,
            tc.tile_pool(name="work", bufs=4) as wpool,
            tc.tile_pool(name="psm", bufs=5, space="PSUM") as psm,
            tc.tile_pool(name="psy", bufs=2, space="PSUM") as psy,
        ):
            # persistent constants in SBUF
            idx_sb = cpool.tile([128, NT * 8], i16)
            nc.sync.dma_start(idx_sb[:], idx_d[:])
            dl_sb = cpool.tile([128, NT, 2], bf16)
            nc.sync.dma_start(dl_sb[:], dl_d[:].rearrange("p (t r) -> p t r", r=2))
            disc_sb = cpool.tile([128, NB], f32)
            nc.sync.dma_start(disc_sb[:], disc_d[:])
            disr_sb = cpool.tile([128, NB * BLK], bf16)
            nc.sync.dma_start(disr_sb[:], disr_d[:])
            iota_sb = cpool.tile([128, 128], bf16)
            nc.sync.dma_start(iota_sb[:], iota_d[:])
            ident_sb = cpool.tile([128, 128], bf16)
            nc.sync.dma_start(ident_sb[:], ident_d[:])
            w_sb = []
            for l in range(NL):
                wt = cpool.tile([F, F], bf16, name=f"w{l}_sb")
                nc.sync.dma_start(wt[:], w_d[l][:])
                w_sb.append(wt)
            wout_sb = cpool.tile([F, 1], bf16)
            nc.sync.dma_start(wout_sb[:], wout_d[:])
            boutc_sb = cpool.tile([128, 1], f32)
            nc.sync.dma_start(boutc_sb[:], boutc_d[:])
            bb_sb = []
            if has_bias:
                for l in range(NL):
                    bt = cpool.tile([128, F], f32, name=f"bb{l}_sb")
                    nc.sync.dma_start(bt[:], bb_d[l][:])
                    bb_sb.append(bt)

            for rep in range(reps):
              for l in range(NL + 1):
                last = l == NL
                table = xt0_d if l == 0 else full[l - 1]
                for bl in range(NB):
                    ncols = min(BLK, P - bl * BLK)
                    rows = slice(bl * BLK, bl * BLK + ncols)
                    tiles_bl = T[bl][0] + T[bl][1]

                    # transposed accumulator psT[f, d]; self-loop term first
                    ps_m = psm.tile([128, 128], f32, tag="psm")
                    xl = wpool.tile([128, F], bf16, tag="xl")
                    xl_src = xtl0_d if l == 0 else shard[l - 1]
                    nc.sync.dma_start(xl[:ncols, :], xl_src[rows, :])
                    nc.tensor.matmul(ps_m[:], xl[:], ident_sb[:],
                                     start=True, stop=(abl == "nomm"))

                    ti = 0
                    for h in (0, 1):
                        Th = T[bl][h]
                        if Th == 0:
                            continue
                        o = off[bl][h]
                        for c0 in range(0, Th, GCAP):
                            cn = min(GCAP, Th - c0)
                            oc = o + c0
                            msg = mpool.tile([128, GCAP, F], bf16, tag="msg")
                            if abl != "nogather":
                                nc.gpsimd.dma_gather(
                                    msg[:, :cn, :],
                                    table[hb[h]:hb[h + 1], :],
                                    idx_sb[:, oc * 8:(oc + cn) * 8],
                                    cn * 128, cn * 128, F,
                                )
                            # one-hot for all cn tiles in ONE DVE op:
                            # oh[p, t, j] = (iota[p, j] == dl[p, oc+t]).
                            # All operands keep an innermost packed
                            # [stride 1, count 2] dim so the DVE 2x_1p
                            # fast mode applies (j split as 64x2, dl
                            # pair-duplicated on host).
                            oh = ohpool.tile([128, GCAP, 128], bf16, tag="oh")
                            if abl != "noonehot":
                              nc.vector.tensor_tensor(
                                oh[:, :cn, :]
                                    .rearrange("p c (k r) -> p c k r", r=2),
                                iota_sb[:, :]
                                    .rearrange("p (k r) -> p k r", r=2)
                                    .unsqueeze(1)
                                    .broadcast_to([128, cn, 64, 2]),
                                dl_sb[:, oc:oc + cn, :].unsqueeze(2)
                                    .broadcast_to([128, cn, 64, 2]),
                                eq)
                            for t in range(cn):
                                if abl != "nomm":
                                    nc.tensor.matmul(
                                        ps_m[:], msg[:, t, :], oh[:, t, :],
                                        start=False,
                                        stop=(ti == tiles_bl - 1))
                                ti += 1

                    # epilogue: uT[f,d] = psT * dis[d]  (free-dim scale)
                    ut = wpool.tile([128, 128], bf16, tag="ut")
                    nc.vector.tensor_tensor(
                        ut[:, :ncols], ps_m[:, :ncols],
                        disr_sb[:, bl * BLK:bl * BLK + ncols], mult)

                    fo = 1 if last else F
                    ps_y = psy.tile([128, 128], f32, tag="psy")
                    nc.tensor.matmul(
                        ps_y[:ncols, :fo], ut[:, :ncols],
                        (wout_sb[:, :] if last else w_sb[l][:, :]),
                        start=True, stop=True)

                    if last:
                        ys = wpool.tile([128, F], f32, tag="ysf")
                        nc.scalar.activation(
                            ys[:ncols, :1], ps_y[:ncols, :1], Relu)
                        nc.sync.dma_start(out_d[rows, :], ys[:ncols, :1])
                    else:
                        ys = wpool.tile([128, F], bf16, tag="ys")
                        if has_bias:
                            yb = wpool.tile([128, F], f32, tag="yb")
                            nc.vector.tensor_tensor(
                                yb[:ncols, :], ps_y[:ncols, :],
                                bb_sb[l][:ncols, :], add)
                            nc.scalar.activation(
                                ys[:ncols, :], yb[:ncols, :], Relu)
                        else:
                            nc.scalar.activation(
                                ys[:ncols, :], ps_y[:ncols, :], Relu)
                        xn = wpool.tile([128, F], bf16, tag="xn")
                        nc.vector.tensor_scalar(
                            xn[:ncols, :], ys[:ncols, :],
                            disc_sb[:ncols, bl:bl + 1], None, mult)
                        nc.sync.dma_start(shard[l][rows, :], xn[:ncols, :])

                        # chunk finished -> AllGather its rows now so the
                        # collective overlaps the remaining blocks' compute
                        for k in range(nch):
                            if bl == bl_start[k + 1] - 1:
                                r0, r1 = row_start[k], row_start[k + 1]
                                g0 = gbase[k]
                                g1 = g0 + (r1 - r0) * C
                                if abl != "noag":
                                    nc.gpsimd.collective_compute(
                                        "AllGather", mybir.AluOpType.bypass,
                                        replica_groups=[list(range(C))],
                                        ins=[shard[l][r0:r1, :]],
                                        outs=[full[l][g0:g1, :]])

    nc.compile()
    return nc


# --------------------------------------------------------------------------
# Entry points
# --------------------------------------------------------------------------

_CACHE = {}


def _get_compiled(plan):
    import os
    key = os.environ.get("KABL", "") + repr(sorted(plan.items()))
    if key not in _CACHE:
        _CACHE[key] = _build(plan)
    return _CACHE[key]


def run(inputs, trace=False):
    """Full pipeline; returns (output [N,1] f32, BassKernelResults)."""
    from concourse.bass_utils import run_bass_kernel_spmd

    plan, in_maps = _preprocess(**inputs)
    nc = _get_compiled(plan)
    res = run_bass_kernel_spmd(nc, in_maps, list(range(plan["C"])),
                               trace=trace)
    out = np.concatenate(
        [res.results[i]["out"] for i in range(plan["C"])], axis=0)
    return out.astype(np.float32), res


def _sharded_runner(nc, C, donate=True):
    """Build a jitted shard_map callable for a compiled Bacc program.
    Returns (fn, in_names, out_names, out_avals)."""
    import jax
    from jax.sharding import Mesh, PartitionSpec
    from jax.experimental.shard_map import shard_map
    import concourse.mybir as mybir
    from concourse import bass2jax
    from concourse.bass2jax import _bass_exec_p, partition_id_tensor

    bass2jax.install_neuronx_cc_hook()
    partition_name = (nc.partition_id_tensor.name
                      if nc.partition_id_tensor else None)
    in_names, out_names, out_avals = [], [], []
    for alloc in nc.m.functions[0].allocations:
        if not isinstance(alloc, mybir.MemoryLocationSet):
            continue
        name = alloc.memorylocations[0].name
        if alloc.kind == "ExternalInput":
            if name != partition_name:
                in_names.append(name)
        elif alloc.kind == "ExternalOutput":
            out_names.append(name)
            out_avals.append(jax.core.ShapedArray(
                tuple(alloc.tensor_shape), mybir.dt.np(alloc.dtype)))
    n_params = len(in_names)
    n_outs = len(out_avals)
    all_in_names = tuple(in_names + out_names +
                         ([partition_name] if partition_name else []))

    def _body(*args):
        operands = list(args)
        if partition_name is not None:
            operands.append(partition_id_tensor())
        outs = _bass_exec_p.bind(
            *operands,
            out_avals=tuple(out_avals),
            in_names=all_in_names,
            out_names=tuple(out_names),
            lowering_input_output_aliases=(),
            sim_require_finite=True,
            sim_require_nnan=True,
            nc=nc,
        )
        return tuple(outs)

    devices = jax.devices()[:C]
    mesh = Mesh(np.array(devices), ("core",))
    in_specs = (PartitionSpec("core"),) * (n_params + n_outs)
    out_specs = (PartitionSpec("core"),) * n_outs
    fn = jax.jit(shard_map(_body, mesh=mesh, in_specs=in_specs,
                           out_specs=out_specs, check_rep=False),
                 donate_argnums=(tuple(range(n_params, n_params + n_outs))
                                 if donate else ()),
                 keep_unused=True)
    return fn, mesh, in_names, out_names, out_avals


def _time_runner(fn, mesh, dev_in, zero_shapes, reps):
    import time
    import jax

    best = float("inf")
    outs = None
    for _ in range(reps):
        zeros = [np.zeros(s, d) for s, d in zero_shapes]
        t0 = time.perf_counter()
        outs = fn(*dev_in, *zeros)
        jax.block_until_ready(outs)
        best = min(best, time.perf_counter() - t0)
    return best, outs


def _floor_runner(C):
    """Build a trivial 8-core bass NEFF runner through the same path."""
    from concourse import bacc, tile
    import concourse.mybir as mybir
    import jax
    from jax.sharding import NamedSharding, PartitionSpec

    nc = bacc.Bacc("TRN2", debug=False, num_devices=C)
    a_d = nc.dram_tensor("a", [128, 128], mybir.dt.float32,
                         kind="ExternalInput")
    o_d = nc.dram_tensor("o", [128, 128], mybir.dt.float32,
                         kind="ExternalOutput")
    with tile.TileContext(nc) as tc:
        with tc.tile_pool(name="p", bufs=1) as p:
            t = p.tile([128, 128], mybir.dt.float32)
            nc.sync.dma_start(t[:], a_d[:])
            nc.sync.dma_start(o_d[:], t[:])
    nc.compile()
    fn, mesh, in_names, out_names, out_avals = _sharded_runner(nc, C)
    a = np.zeros((C * 128, 128), np.float32)
    dev_in = [jax.device_put(a, NamedSharding(mesh, PartitionSpec("core")))]
    zero_shapes = [((C * 128, 128), np.float32)]
    return fn, mesh, dev_in, zero_shapes


def bench(inputs, iters=12):
    """Estimate on-device exec time: min wall time of the kernel NEFF with
    device-resident inputs, minus the dispatch floor of a trivial NEFF.
    Kernel and floor calls are interleaved in time so that dispatch-latency
    drift affects both equally.  Returns (output [N,1], est_exec_ns)."""
    import time
    import jax
    from jax.sharding import NamedSharding, PartitionSpec

    plan, in_maps = _preprocess(**inputs)
    C = plan["C"]
    nc = _get_compiled(plan)
    fn, mesh, in_names, out_names, out_avals = _sharded_runner(nc, C)

    concat_in = [np.concatenate([np.asarray(m[nm]) for m in in_maps], axis=0)
                 for nm in in_names]
    sh = NamedSharding(mesh, PartitionSpec("core"))
    dev_in = [jax.device_put(a, sh) for a in concat_in]
    zero_shapes = [((C * a.shape[0], *a.shape[1:]), a.dtype)
                   for a in out_avals]
    zeros = [np.zeros(s, d) for s, d in zero_shapes]
    jax.block_until_ready(fn(*dev_in, *zeros))  # warmup/compile

    ffn, fmesh, fdev_in, fzero_shapes = _floor_runner(C)
    fzeros = [np.zeros(s, d) for s, d in fzero_shapes]
    jax.block_until_ready(ffn(*fdev_in, *fzeros))  # warmup/compile

    best = floor = float("inf")
    outs = None
    for _ in range(iters):
        zeros = [np.zeros(s, d) for s, d in zero_shapes]
        t0 = time.perf_counter()
        outs = fn(*dev_in, *zeros)
        jax.block_until_ready(outs)
        best = min(best, time.perf_counter() - t0)

        fzeros = [np.zeros(s, d) for s, d in fzero_shapes]
        t0 = time.perf_counter()
        fouts = ffn(*fdev_in, *fzeros)
        jax.block_until_ready(fouts)
        floor = min(floor, time.perf_counter() - t0)
    est_ns = max(best - floor, 0.0) * 1e9
    print(f"[bench] kernel call min {best*1e3:.3f} ms, "
          f"dispatch floor {floor*1e3:.3f} ms")
    oi = out_names.index("out")
    out = np.asarray(outs[oi]).reshape(C, -1, 1).reshape(-1, 1)
    return out.astype(np.float32), est_ns


def kernel(**inputs):
    out, _ = run(inputs, trace=False)
    return out



# revision 36
# speedup vs baseline: 6.0634x; 6.0634x over previous
"""GCN (CapsGNN) message-passing kernel for 8 Trainium2 NeuronCores.

Algorithm (mathematically identical to the reference):
    deg[i] = 1 + indeg(i);  dis = deg**-0.5
    With xt := dis * x (row-scaled activations), each layer is
        y = relu( dis[d] * ( sum_{e: dst=d} xt[src_e]  +  xt[d] ) @ W + b )
    because norm(e) = dis[src]*dis[dst] is separable and matmul is linear.

Distribution: nodes are sharded contiguously across 8 cores (core c owns
rows [c*P, (c+1)*P)); edges are partitioned by destination shard so the
segment-sum is local.  Tables and all matmul operands are bf16 (validated
rel err ~8e-3 vs the 2e-2 gate).  Per layer each core:
  1. dma_gather's xt[src] rows (256B bf16 each) for its edges from a full
     replicated table in HBM (int16 gather indices -> table is addressed
     in two chunk-aligned halves of < 32768 rows each),
  2. accumulates the segment-sum TRANSPOSED in PSUM: for each 128-edge
     tile, matmul(psT[f,d] += msg[e,f].T @ onehot[e,d]) with on-chip
     one-hot matrices (iota == dst_local), plus one identity matmul that
     adds the self-loop term xt_local.T -- so no per-block transpose is
     needed before the weight matmul,
  3. epilogue per 128-dst block: uT = psT * disrow (free-dim dis scale,
     one DVE op), y = uT.T @ W (PE), relu (ACT), xn = y * dis (DVE),
  4. writes its shard of the next table and AllGathers it across cores.

Host-side preprocessing (free): degree computation, edge sorting/padding
by (dst block, src half, src), gather indices, dst-local labels.
"""

import math
import numpy as np

N_CORES = 8
F = 128  # feature width of every hidden layer (== partition count)
BLK = 128  # dst nodes per aggregation block
# Max 128-idx tiles per dma_gather call: the SWDGE ring is hard-capped at
# 1024 descriptors per call (GCAP=10/16/18 hang the device; scratch size
# does not change it).
GCAP = 8
DMA_SCRATCH = 16384


# --------------------------------------------------------------------------
# Host-side preprocessing
# --------------------------------------------------------------------------

def _preprocess(features, W, b, W_out, b_out, edges):
    import ml_dtypes
    bf16 = ml_dtypes.bfloat16

    features = np.asarray(features, dtype=np.float32)
    W = np.asarray(W, dtype=np.float32)
    b = np.asarray(b, dtype=np.float32)
    W_out = np.asarray(W_out, dtype=np.float32)
    b_out = np.asarray(b_out, dtype=np.float32)
    edges = np.asarray(edges)

    N = features.shape[0]
    C = N_CORES
    assert N % C == 0
    P = N // C
    NB = (P + BLK - 1) // BLK
    E = edges.shape[1]

    src = edges[0].astype(np.int64)
    dst = edges[1].astype(np.int64)

    deg = (np.bincount(dst, minlength=N).astype(np.float32) + np.float32(1.0))
    dis = (deg ** np.float32(-0.5)).astype(np.float32)
    xt0 = (features * dis[:, None]).astype(bf16)

    core = dst // P
    blk = (dst % P) // BLK
    dloc = ((dst % P) % BLK).astype(np.float32)

    # ---- chunk-major table layout -------------------------------------
    # The full gather table is stored chunk-major: for chunk k (a range of
    # dst blocks), all cores' rows of that chunk are contiguous.  This lets
    # the per-layer AllGather run as NCHUNK slice-collectives, each firing
    # as soon as its blocks' outputs are written (overlapped with compute).
    nch = 4
    base_bl = NB // nch
    extra = NB - base_bl * nch
    chunk_blocks = [base_bl + (1 if k < extra else 0) for k in range(nch)]
    bl_start = np.concatenate([[0], np.cumsum(chunk_blocks)]).astype(np.int64)
    row_start = np.minimum(bl_start * BLK, P)  # per-core local row ranges
    chunk_rows = np.diff(row_start)            # rows per core per chunk
    gbase = np.concatenate([[0], np.cumsum(chunk_rows * C)]).astype(np.int64)

    rloc_all = np.arange(N, dtype=np.int64) % P
    core_all = np.arange(N, dtype=np.int64) // P
    k_all = np.searchsorted(row_start[1:], rloc_all, side="right")
    posv = gbase[k_all] + core_all * chunk_rows[k_all] + (rloc_all - row_start[k_all])
    xt0_perm = np.empty_like(xt0)
    xt0_perm[posv] = xt0

    cs = int(gbase[2])  # first-two-chunks size; both halves fit int16
    assert cs < 32768 and N - cs < 32768
    psrc = posv[src]
    half = (psrc >= cs).astype(np.int64)
    idxv = (psrc - half * cs).astype(np.int16)

    # group id per edge: (core, blk, half)
    gid = (core * NB + blk) * 2 + half
    cnt = np.bincount(gid, minlength=C * NB * 2).reshape(C, NB, 2)
    Tneed = -(-cnt // BLK)  # ceil division -> tiles needed per group
    T = Tneed.max(axis=0)  # [NB, 2] same tile counts on every core (SPMD)
    for bl in range(NB):
        if T[bl].sum() == 0:
            T[bl, 0] = 1  # keep at least one (all-padding) tile per block

    off = np.zeros((NB, 2), np.int64)
    o = 0
    for bl in range(NB):
        for h in (0, 1):
            off[bl, h] = o
            o += T[bl, h]
    NT = int(o)

    # slab position for every edge: off[blk,half]*128 + within-group rank.
    # Secondary sort by src position keeps gather addresses ascending within
    # a group (HBM row locality).
    order = np.lexsort((psrc, half, blk, core))
    counts_flat = np.bincount(gid, minlength=C * NB * 2)
    starts = np.zeros(C * NB * 2, np.int64)
    np.cumsum(counts_flat[:-1], out=starts[1:])
    rank = np.arange(E, dtype=np.int64) - starts[gid[order]]
    soff = off[blk[order], half[order]] * BLK + rank

    idx_slab = np.zeros((C, NT * BLK), np.int16)
    dl_slab = np.full((C, NT * BLK), -1.0, np.float32)  # cast bf16 below
    cc = core[order]
    idx_slab[cc, soff] = idxv[order]
    dl_slab[cc, soff] = dloc[order]

    iota = np.ascontiguousarray(
        np.broadcast_to(np.arange(BLK, dtype=np.float32), (BLK, BLK))
    ).astype(bf16)
    ident = np.eye(BLK, dtype=np.float32).astype(bf16)
    has_bias = bool(np.any(b != 0.0))
    NL = W.shape[0]  # stacked hidden layers (3)

    # ---- final (128->1) layer: fixed-K scalar slot table ------------------
    # out[d] = relu(dis_d * (sum_{e->d} s[src_e] + s[d]) + b_out) with
    # s[n] = xt3[n] @ W_out, so the last layer only needs per-edge SCALARS.
    # s (bf16, [N]) is AllGather'd (200KB) and replicated across SBUF
    # partitions; per dst-block a single gpsimd indirect_copy fetches
    # s[src] for every edge into a [128, 16, K] slab (lane p sums range
    # p%16 of its 16-partition group), padded per dst to the global max
    # in-degree K with pointers to a zero slot.
    indeg = (deg - np.float32(1.0)).astype(np.int64)
    KSLOT = int(indeg.max())
    order2 = np.lexsort((src, dst))
    dsto = dst[order2]
    srco = src[order2]
    starts2 = np.searchsorted(dsto, np.arange(N, dtype=np.int64))
    rank2 = np.arange(E, dtype=np.int64) - starts2[dsto]
    assert rank2.max() < KSLOT
    c2 = dsto // P
    pl2 = dsto % P
    bl2 = pl2 // BLK
    dl2 = pl2 % BLK
    i2 = (dl2 % 16) * KSLOT + rank2      # slot within the 16-lane group
    g2 = dl2 // 16
    idxf = np.full((C, 128, NB * KSLOT), N, np.int64)  # pad -> zero slot N
    idxf[c2, 16 * g2 + (i2 % 16), bl2 * KSLOT + i2 // 16] = srco
    ddiag = (np.arange(16)[None, :] ==
             (np.arange(128) % 16)[:, None]).astype(np.float32)

    plan = dict(N=N, C=C, P=P, NB=NB, NT=NT, CS=cs, NL=NL, KSLOT=KSLOT,
                T=T.tolist(), off=off.tolist(),
                Tmax=int(T.max()), has_bias=has_bias,
                bl_start=bl_start.tolist(), row_start=row_start.tolist(),
                gbase=gbase.tolist(), nch=nch)

    in_maps = []
    for c in range(C):
        dis_c = np.zeros(NB * BLK, np.float32)
        dis_c[:P] = dis[c * P:(c + 1) * P]
        m = {
            "xt0": xt0_perm,
            "xtl0": np.ascontiguousarray(xt0[c * P:(c + 1) * P]),
            "idx": np.ascontiguousarray(
                np.tile(idx_slab[c].reshape(NT * 8, 16).T, (8, 1))),
            # pair-duplicated dstloc: dld[p, 2t+r] = dloc of edge (t, p),
            # r=0,1.  The duplicate lets the batched one-hot comparison keep
            # a [stride 1, count 2] innermost AP dim (DVE 2x_1p fast mode).
            "dstloc": np.ascontiguousarray(np.repeat(
                dl_slab[c].reshape(NT, BLK).T, 2, axis=1)).astype(bf16),
            "discol": np.ascontiguousarray(dis_c.reshape(NB, BLK).T),
            # dis broadcast down partitions: [128, NB*128] for free-dim scale
            "disrow": np.ascontiguousarray(
                np.broadcast_to(dis_c[None, :], (BLK, NB * BLK))).astype(bf16),
            "iota": iota,
            "ident": ident,
            "wout": np.ascontiguousarray(W_out).astype(bf16),
            "boutc": np.full((BLK, 1), b_out[0], np.float32),
        }
        for l in range(NL):
            m[f"w{l}"] = np.ascontiguousarray(W[l]).astype(bf16)
            if has_bias:
                m[f"bb{l}"] = np.ascontiguousarray(
                    np.broadcast_to(b[l][None, :], (BLK, F))).astype(np.float32)
        in_maps.append(m)
    return plan, in_maps


# --------------------------------------------------------------------------
# Bass/Tile kernel builder
# --------------------------------------------------------------------------

def _build(plan):
    import os
    from concourse import bacc, tile
    import concourse.mybir as mybir

    abl = os.environ.get("KABL", "")  # ablation probe: noag/nogather/noonehot/nomm
    reps = int(plan.get("reps", 1))  # whole-network repetitions (timing slope)

    N, C, P, NB, NT, CS, NL, KSLOT = (plan[k] for k in
                                      ("N", "C", "P", "NB", "NT", "CS", "NL",
                                       "KSLOT"))
    hb = [0, CS, N]  # gather-table half bounds == chunk collective regions
    T, off, Tmax, has_bias = (plan[k] for k in
                              ("T", "off", "Tmax", "has_bias"))
    bl_start, row_start, gbase, nch = (plan[k] for k in
                                       ("bl_start", "row_start", "gbase",
                                        "nch"))
    f32 = mybir.dt.float32
    bf16 = mybir.dt.bfloat16
    i16 = mybir.dt.int16
    u16 = mybir.dt.uint16
    Relu = mybir.ActivationFunctionType.Relu
    eq = mybir.AluOpType.is_equal
    add = mybir.AluOpType.add
    mult = mybir.AluOpType.mult

    nc = bacc.Bacc("TRN2", debug=False, num_devices=C,
                   target_bir_lowering=False,
                   dynamic_dma_scratch_size=DMA_SCRATCH,
                   num_swdge_queues=2)

    xt0_d = nc.dram_tensor("xt0", [N, F], bf16, kind="ExternalInput")
    xtl0_d = nc.dram_tensor("xtl0", [P, F], bf16, kind="ExternalInput")
    idx_d = nc.dram_tensor("idx", [128, NT * 8], i16, kind="ExternalInput")
    dl_d = nc.dram_tensor("dstloc", [128, NT * 2], bf16, kind="ExternalInput")
    disc_d = nc.dram_tensor("discol", [128, NB], f32, kind="ExternalInput")
    disr_d = nc.dram_tensor("disrow", [128, NB * BLK], bf16,
                            kind="ExternalInput")
    iota_d = nc.dram_tensor("iota", [128, 128], bf16, kind="ExternalInput")
    ident_d = nc.dram_tensor("ident", [128, 128], bf16, kind="ExternalInput")
    w_d = [nc.dram_tensor(f"w{l}", [F, F], bf16, kind="ExternalInput")
           for l in range(NL)]
    wout_d = nc.dram_tensor("wout", [F, 1], bf16, kind="ExternalInput")
    boutc_d = nc.dram_tensor("boutc", [128, 1], f32, kind="ExternalInput")
    bb_d = [nc.dram_tensor(f"bb{l}", [128, F], f32, kind="ExternalInput")
            for l in range(NL)] if has_bias else None
    out_d = nc.dram_tensor("out", [P, 1], f32, kind="ExternalOutput")

    shard = [nc.dram_tensor(f"xsh{l}", [P, F], bf16) for l in range(NL)]
    full = [nc.dram_tensor(f"xfull{l}", [N, F], bf16, addr_space="Shared")
            for l in range(NL)]

    with tile.TileContext(nc) as tc:
        with (
            tc.tile_pool(name="const", bufs=1) as cpool,
            tc.tile_pool(name="msg", bufs=4) as mpool,
            tc.tile_pool(name="oh", bufs=4) as ohpool,
            tc.tile_pool(name="work", bufs=4) as wpool,
            tc.tile_pool(name="psm", bufs=5, space="PSUM") as psm,
            tc.tile_pool(name="psy", bufs=2, space="PSUM") as psy,
        ):
            # persistent constants in SBUF
            idx_sb = cpool.tile([128, NT * 8], i16)
            nc.sync.dma_start(idx_sb[:], idx_d[:])
            dl_sb = cpool.tile([128, NT, 2], bf16)
            nc.sync.dma_start(dl_sb[:], dl_d[:].rearrange("p (t r) -> p t r", r=2))
            disc_sb = cpool.tile([128, NB], f32)
            nc.sync.dma_start(disc_sb[:], disc_d[:])
            disr_sb = cpool.tile([128, NB * BLK], bf16)
            nc.sync.dma_start(disr_sb[:], disr_d[:])
            iota_sb = cpool.tile([128, 128], bf16)
            nc.sync.dma_start(iota_sb[:], iota_d[:])
            ident_sb = cpool.tile([128, 128], bf16)
            nc.sync.dma_start(ident_sb[:], ident_d[:])
            w_sb = []
            for l in range(NL):
                wt = cpool.tile([F, F], bf16, name=f"w{l}_sb")
                nc.sync.dma_start(wt[:], w_d[l][:])
                w_sb.append(wt)
            wout_sb = cpool.tile([F, 1], bf16)
            nc.sync.dma_start(wout_sb[:], wout_d[:])
            boutc_sb = cpool.tile([128, 1], f32)
            nc.sync.dma_start(boutc_sb[:], boutc_d[:])
            bb_sb = []
            if has_bias:
                for l in range(NL):
                    bt = cpool.tile([128, F], f32, name=f"bb{l}_sb")
                    nc.sync.dma_start(bt[:], bb_d[l][:])
                    bb_sb.append(bt)

            for rep in range(reps):
              for l in range(NL + 1):
                last = l == NL
                table = xt0_d if l == 0 else full[l - 1]
                for bl in range(NB):
                    ncols = min(BLK, P - bl * BLK)
                    rows = slice(bl * BLK, bl * BLK + ncols)
                    tiles_bl = T[bl][0] + T[bl][1]

                    # transposed accumulator psT[f, d]; self-loop term first
                    ps_m = psm.tile([128, 128], f32, tag="psm")
                    xl = wpool.tile([128, F], bf16, tag="xl")
                    xl_src = xtl0_d if l == 0 else shard[l - 1]
                    nc.sync.dma_start(xl[:ncols, :], xl_src[rows, :])
                    nc.tensor.matmul(ps_m[:], xl[:], ident_sb[:],
                                     start=True, stop=(abl == "nomm"))

                    ti = 0
                    qn = 0
                    for h in (0, 1):
                        Th = T[bl][h]
                        if Th == 0:
                            continue
                        o = off[bl][h]
                        for c0 in range(0, Th, GCAP):
                            cn = min(GCAP, Th - c0)
                            oc = o + c0
                            msg = mpool.tile([128, GCAP, F], bf16, tag="msg")
                            if abl != "nogather":
                                nc.gpsimd.dma_gather(
                                    msg[:, :cn, :],
                                    table[hb[h]:hb[h + 1], :],
                                    idx_sb[:, oc * 8:(oc + cn) * 8],
                                    cn * 128, cn * 128, F,
                                    queue_num=qn,
                                )
                                qn = 1 - qn
                            # one-hot for all cn tiles in ONE DVE op:
                            # oh[p, t, j] = (iota[p, j] == dl[p, oc+t]).
                            # All operands keep an innermost packed
                            # [stride 1, count 2] dim so the DVE 2x_1p
                            # fast mode applies (j split as 64x2, dl
                            # pair-duplicated on host).
                            oh = ohpool.tile([128, GCAP, 128], bf16, tag="oh")
                            if abl != "noonehot":
                              nc.vector.tensor_tensor(
                                oh[:, :cn, :]
                                    .rearrange("p c (k r) -> p c k r", r=2),
                                iota_sb[:, :]
                                    .rearrange("p (k r) -> p k r", r=2)
                                    .unsqueeze(1)
                                    .broadcast_to([128, cn, 64, 2]),
                                dl_sb[:, oc:oc + cn, :].unsqueeze(2)
                                    .broadcast_to([128, cn, 64, 2]),
                                eq)
                            for t in range(cn):
                                if abl != "nomm":
                                    nc.tensor.matmul(
                                        ps_m[:], msg[:, t, :], oh[:, t, :],
                                        start=False,
                                        stop=(ti == tiles_bl - 1))
                                ti += 1

                    # epilogue: uT[f,d] = psT * dis[d]  (free-dim scale)
                    ut = wpool.tile([128, 128], bf16, tag="ut")
                    nc.vector.tensor_tensor(
                        ut[:, :ncols], ps_m[:, :ncols],
                        disr_sb[:, bl * BLK:bl * BLK + ncols], mult)

                    fo = 1 if last else F
                    ps_y = psy.tile([128, 128], f32, tag="psy")
                    nc.tensor.matmul(
                        ps_y[:ncols, :fo], ut[:, :ncols],
                        (wout_sb[:, :] if last else w_sb[l][:, :]),
                        start=True, stop=True)

                    if last:
                        ys = wpool.tile([128, F], f32, tag="ysf")
                        nc.scalar.activation(
                            ys[:ncols, :1], ps_y[:ncols, :1], Relu)
                        nc.sync.dma_start(out_d[rows, :], ys[:ncols, :1])
                    else:
                        ys = wpool.tile([128, F], bf16, tag="ys")
                        if has_bias:
                            yb = wpool.tile([128, F], f32, tag="yb")
                            nc.vector.tensor_tensor(
                                yb[:ncols, :], ps_y[:ncols, :],
                                bb_sb[l][:ncols, :], add)
                            nc.scalar.activation(
                                ys[:ncols, :], yb[:ncols, :], Relu)
                        else:
                            nc.scalar.activation(
                                ys[:ncols, :], ps_y[:ncols, :], Relu)
                        xn = wpool.tile([128, F], bf16, tag="xn")
                        nc.vector.tensor_scalar(
                            xn[:ncols, :], ys[:ncols, :],
                            disc_sb[:ncols, bl:bl + 1], None, mult)
                        nc.sync.dma_start(shard[l][rows, :], xn[:ncols, :])

                        # chunk finished -> AllGather its rows now so the
                        # collective overlaps the remaining blocks' compute
                        for k in range(nch):
                            if bl == bl_start[k + 1] - 1:
                                r0, r1 = row_start[k], row_start[k + 1]
                                g0 = gbase[k]
                                g1 = g0 + (r1 - r0) * C
                                if abl != "noag":
                                    nc.gpsimd.collective_compute(
                                        "AllGather", mybir.AluOpType.bypass,
                                        replica_groups=[list(range(C))],
                                        ins=[shard[l][r0:r1, :]],
                                        outs=[full[l][g0:g1, :]])

    nc.compile()
    return nc


# --------------------------------------------------------------------------
# Entry points
# --------------------------------------------------------------------------

_CACHE = {}


def _get_compiled(plan):
    import os
    key = os.environ.get("KABL", "") + repr(sorted(plan.items()))
    if key not in _CACHE:
        _CACHE[key] = _build(plan)
    return _CACHE[key]


def run(inputs, trace=False):
    """Full pipeline; returns (output [N,1] f32, BassKernelResults)."""
    from concourse.bass_utils import run_bass_kernel_spmd

    plan, in_maps = _preprocess(**inputs)
    nc = _get_compiled(plan)
    res = run_bass_kernel_spmd(nc, in_maps, list(range(plan["C"])),
                               trace=trace)
    out = np.concatenate(
        [res.results[i]["out"] for i in range(plan["C"])], axis=0)
    return out.astype(np.float32), res


def _sharded_runner(nc, C, donate=True):
    """Build a jitted shard_map callable for a compiled Bacc program.
    Returns (fn, in_names, out_names, out_avals)."""
    import jax
    from jax.sharding import Mesh, PartitionSpec
    from jax.experimental.shard_map import shard_map
    import concourse.mybir as mybir
    from concourse import bass2jax
    from concourse.bass2jax import _bass_exec_p, partition_id_tensor

    bass2jax.install_neuronx_cc_hook()
    partition_name = (nc.partition_id_tensor.name
                      if nc.partition_id_tensor else None)
    in_names, out_names, out_avals = [], [], []
    for alloc in nc.m.functions[0].allocations:
        if not isinstance(alloc, mybir.MemoryLocationSet):
            continue
        name = alloc.memorylocations[0].name
        if alloc.kind == "ExternalInput":
            if name != partition_name:
                in_names.append(name)
        elif alloc.kind == "ExternalOutput":
            out_names.append(name)
            out_avals.append(jax.core.ShapedArray(
                tuple(alloc.tensor_shape), mybir.dt.np(alloc.dtype)))
    n_params = len(in_names)
    n_outs = len(out_avals)
    all_in_names = tuple(in_names + out_names +
                         ([partition_name] if partition_name else []))

    def _body(*args):
        operands = list(args)
        if partition_name is not None:
            operands.append(partition_id_tensor())
        outs = _bass_exec_p.bind(
            *operands,
            out_avals=tuple(out_avals),
            in_names=all_in_names,
            out_names=tuple(out_names),
            lowering_input_output_aliases=(),
            sim_require_finite=True,
            sim_require_nnan=True,
            nc=nc,
        )
        return tuple(outs)

    devices = jax.devices()[:C]
    mesh = Mesh(np.array(devices), ("core",))
    in_specs = (PartitionSpec("core"),) * (n_params + n_outs)
    out_specs = (PartitionSpec("core"),) * n_outs
    fn = jax.jit(shard_map(_body, mesh=mesh, in_specs=in_specs,
                           out_specs=out_specs, check_rep=False),
                 donate_argnums=(tuple(range(n_params, n_params + n_outs))
                                 if donate else ()),
                 keep_unused=True)
    return fn, mesh, in_names, out_names, out_avals


def _time_runner(fn, mesh, dev_in, zero_shapes, reps):
    import time
    import jax

    best = float("inf")
    outs = None
    for _ in range(reps):
        zeros = [np.zeros(s, d) for s, d in zero_shapes]
        t0 = time.perf_counter()
        outs = fn(*dev_in, *zeros)
        jax.block_until_ready(outs)
        best = min(best, time.perf_counter() - t0)
    return best, outs


def _floor_runner(C):
    """Build a trivial 8-core bass NEFF runner through the same path."""
    from concourse import bacc, tile
    import concourse.mybir as mybir
    import jax
    from jax.sharding import NamedSharding, PartitionSpec

    nc = bacc.Bacc("TRN2", debug=False, num_devices=C)
    a_d = nc.dram_tensor("a", [128, 128], mybir.dt.float32,
                         kind="ExternalInput")
    o_d = nc.dram_tensor("o", [128, 128], mybir.dt.float32,
                         kind="ExternalOutput")
    with tile.TileContext(nc) as tc:
        with tc.tile_pool(name="p", bufs=1) as p:
            t = p.tile([128, 128], mybir.dt.float32)
            nc.sync.dma_start(t[:], a_d[:])
            nc.sync.dma_start(o_d[:], t[:])
    nc.compile()
    fn, mesh, in_names, out_names, out_avals = _sharded_runner(nc, C)
    a = np.zeros((C * 128, 128), np.float32)
    dev_in = [jax.device_put(a, NamedSharding(mesh, PartitionSpec("core")))]
    zero_shapes = [((C * 128, 128), np.float32)]
    return fn, mesh, dev_in, zero_shapes


def bench(inputs, iters=12):
    """Estimate on-device exec time: min wall time of the kernel NEFF with
    device-resident inputs, minus the dispatch floor of a trivial NEFF.
    Kernel and floor calls are interleaved in time so that dispatch-latency
    drift affects both equally.  Returns (output [N,1], est_exec_ns)."""
    import time
    import jax
    from jax.sharding import NamedSharding, PartitionSpec

    plan, in_maps = _preprocess(**inputs)
    C = plan["C"]
    nc = _get_compiled(plan)
    fn, mesh, in_names, out_names, out_avals = _sharded_runner(nc, C)

    concat_in = [np.concatenate([np.asarray(m[nm]) for m in in_maps], axis=0)
                 for nm in in_names]
    sh = NamedSharding(mesh, PartitionSpec("core"))
    dev_in = [jax.device_put(a, sh) for a in concat_in]
    zero_shapes = [((C * a.shape[0], *a.shape[1:]), a.dtype)
                   for a in out_avals]
    zeros = [np.zeros(s, d) for s, d in zero_shapes]
    jax.block_until_ready(fn(*dev_in, *zeros))  # warmup/compile

    ffn, fmesh, fdev_in, fzero_shapes = _floor_runner(C)
    fzeros = [np.zeros(s, d) for s, d in fzero_shapes]
    jax.block_until_ready(ffn(*fdev_in, *fzeros))  # warmup/compile

    best = floor = float("inf")
    outs = None
    for _ in range(iters):
        zeros = [np.zeros(s, d) for s, d in zero_shapes]
        t0 = time.perf_counter()
        outs = fn(*dev_in, *zeros)
        jax.block_until_ready(outs)
        best = min(best, time.perf_counter() - t0)

        fzeros = [np.zeros(s, d) for s, d in fzero_shapes]
        t0 = time.perf_counter()
        fouts = ffn(*fdev_in, *fzeros)
        jax.block_until_ready(fouts)
        floor = min(floor, time.perf_counter() - t0)
    est_ns = max(best - floor, 0.0) * 1e9
    print(f"[bench] kernel call min {best*1e3:.3f} ms, "
          f"dispatch floor {floor*1e3:.3f} ms")
    oi = out_names.index("out")
    out = np.asarray(outs[oi]).reshape(C, -1, 1).reshape(-1, 1)
    return out.astype(np.float32), est_ns


def kernel(**inputs):
    out, _ = run(inputs, trace=False)
    return out

